# revision 7
# baseline (speedup 1.0000x reference)
"""Multi-head attention (B=2, N=4096, C=512, H=8) on 8 TRN2 NeuronCores.

Sharding (data + head/tensor parallel, per the problem hint): core i handles
batch b = i//4 and head pair {2*(i%4), 2*(i%4)+1}. Projection weights are
split column-wise over heads (q/k/v) and row-wise for the out projection, so
each core computes its heads' full [N, N] score block, writes its attn slab,
and produces a partial out-projection that the host sum-reduces (+bo) during
unshard.

Per-core pipeline (all matmuls f32r unless noted):
  x -> PE-transpose -> xT chunks -> q/k/v projections
  scores S^T[kpos, qrow] = kT.T @ qT per 128-kpos chunk
  ACT exp (scale=1/8) PSUM -> SBUF bf16 (chunk pairs)
  attn @ v_aug (v with ones column, bf16) -> y^T + rowsums in PSUM
  recip -> PE broadcast -> y^T normalize (+bv)
  PE-transpose exp chunks -> natural [qrow, kpos] bf16 PSUM (8 per bank)
  DVE tensor_scalar normalize -> SBUF bf16 -> cast-DMA (bf16->f32) to DRAM
  out partial = y^T.T @ woT
"""

import numpy as np

import concourse.bass as bass
import concourse.bacc as bacc
import concourse.mybir as mybir
from concourse import tile
from concourse.bass_utils import run_bass_kernel_spmd
from concourse.masks import make_identity

f32 = mybir.dt.float32
f32r = mybir.dt.float32r
bf16 = mybir.dt.bfloat16
AF = mybir.ActivationFunctionType

B, N, C = 2, 4096, 512
H, D = 8, 64
KC = N // 128          # 32 kpos chunks
QTILES = N // 512      # 8 query tiles of 512 rows


def build():
    nc = bacc.Bacc("TRN2", target_bir_lowering=False, debug=False, num_devices=8)

    xq = nc.dram_tensor("xq", [N, C], f32, kind="ExternalInput").ap()
    xk = nc.dram_tensor("xk", [N, C], f32, kind="ExternalInput").ap()
    xv = nc.dram_tensor("xv", [N, C], f32, kind="ExternalInput").ap()
    # wq/wk/wv: [128, C] row slices of the weight (this core's 2 heads).
    # wo: [C, 128] column slice.  biases: [1, 128] slices (bo handled on host).
    wq = nc.dram_tensor("wq", [128, C], f32, kind="ExternalInput").ap()
    wk = nc.dram_tensor("wk", [128, C], f32, kind="ExternalInput").ap()
    wv = nc.dram_tensor("wv", [128, C], f32, kind="ExternalInput").ap()
    wo = nc.dram_tensor("wo", [C, 128], f32, kind="ExternalInput").ap()
    bq = nc.dram_tensor("bq", [1, 128], f32, kind="ExternalInput").ap()
    bk = nc.dram_tensor("bk", [1, 128], f32, kind="ExternalInput").ap()
    bv = nc.dram_tensor("bv", [1, 128], f32, kind="ExternalInput").ap()
    attn_d = nc.dram_tensor("attn", [2, N, N], f32, kind="ExternalOutput").ap()
    out_d = nc.dram_tensor("out", [N, C], f32, kind="ExternalOutput").ap()

    with tile.TileContext(nc) as tc:
        with (
            tc.tile_pool(name="const", bufs=1) as const,
            tc.tile_pool(name="wt", bufs=1) as wtp,
            tc.tile_pool(name="xg", bufs=2) as xgp,
            tc.tile_pool(name="xt", bufs=2) as xtp,
            tc.tile_pool(name="big", bufs=1) as bigp,
            tc.tile_pool(name="es", bufs=18) as esp,
            tc.tile_pool(name="asb", bufs=2) as ap_,
            tc.tile_pool(name="sm", bufs=3) as smp,
            tc.tile_pool(name="ot", bufs=2) as otp,
            tc.tile_pool(name="ps_s", bufs=2, space="PSUM") as ps_s,
            tc.tile_pool(name="ps_y", bufs=1, space="PSUM") as ps_y,
            tc.tile_pool(name="ps_t", bufs=2, space="PSUM") as ps_t,
            tc.tile_pool(name="ps_m", bufs=1, space="PSUM") as ps_m,
        ):
            # ---- constants ----
            ident = const.tile([128, 128], f32)
            make_identity(nc, ident[:, :])
            identb = const.tile([128, 128], bf16)
            nc.vector.tensor_copy(identb[:, :], ident[:, :])
            ones64f = const.tile([1, 64], f32, tag="o64f")
            nc.gpsimd.memset(ones64f[:, :], 1.0)
            ones64 = const.tile([1, 64], f32r, tag="o64r")
            nc.vector.tensor_copy(ones64[:, :], ones64f[:, :])

            # bias slices -> [128, 1] per-partition layout
            bsb = {}
            for name, bap in (("bq", bq), ("bk", bk), ("bv", bv)):
                t = const.tile([128, 1], f32, tag=name)
                nc.sync.dma_start(
                    out=t[:, :], in_=bap.rearrange("a (c p) -> p (a c)", p=128)
                )
                bsb[name] = t

            # ---- weight transposes ----
            # wqT/wkT: [p=c%128, ci, dk 128] f32r;  wvT: same;  woT: [p=ci, coj, 128]
            wqT = wtp.tile([128, 4, 128], f32r, tag="wqT")
            wkT = wtp.tile([128, 4, 128], f32r, tag="wkT")
            wvT = wtp.tile([128, 4, 128], f32r, tag="wvT")
            for wap, dst in ((wq, wqT), (wk, wkT), (wv, wvT)):
                t = xgp.tile([128, 4, 128], f32, tag="xg")
                nc.sync.dma_start(
                    out=t[:, :, :],
                    in_=wap.rearrange("p (cj c) -> p cj c", cj=4),
                )
                pt = ps_s.tile([128, 1024], f32, tag="s")
                for ci in range(4):
                    nc.tensor.transpose(
                        pt[:, ci * 128:(ci + 1) * 128],
                        t[:, ci, :],
                        ident[:, :],
                    )
                nc.vector.tensor_copy(
                    dst[:, :, :], pt[:, 0:512].rearrange("p (a b) -> p a b", a=4)
                )
            # woT[p=ci, coj, 128co]: transpose of wo[co, ci]
            wog = xgp.tile([128, 4, 128], f32, tag="xg")
            nc.sync.dma_start(
                out=wog[:, :, :],
                in_=wo.rearrange("(coj p) c -> p coj c", p=128),
            )
            woT = wtp.tile([128, 4, 128], f32r, tag="woT")
            pt = ps_s.tile([128, 1024], f32, tag="s")
            for coj in range(4):
                nc.tensor.transpose(
                    pt[:, coj * 128:(coj + 1) * 128], wog[:, coj, :], ident[:, :]
                )
            nc.vector.tensor_copy(
                woT[:, :, :], pt[:, 0:512].rearrange("p (a b) -> p a b", a=4)
            )

            # ---- x transposes + projections ----
            kT = bigp.tile([128, N], f32r, tag="kT")
            vsb = bigp.tile([128, KC, 2, 65], bf16, tag="v")
            nc.gpsimd.memset(vsb[:, :, :, 64:65], 1.0)
            qT = bigp.tile([128, N], f32r, tag="qT")
            yT = bigp.tile([128, N], f32r, tag="yT")

            def xpose_group(xap, g):
                """Transpose rows [512g, 512g+512) of x -> xt[p=c%128, ci, 512]."""
                xg = xgp.tile([128, 4, C], f32, tag="xg")
                nc.sync.dma_start(
                    out=xg[:, :, :],
                    in_=xap[g * 512:(g + 1) * 512, :].rearrange(
                        "(j p) c -> p j c", p=128
                    ),
                )
                xt = xtp.tile([128, 4, 512], f32r, tag="xt")
                for cpair in range(2):
                    pt = ps_s.tile([128, 1024], f32, tag="s")
                    for half in range(2):
                        ci = cpair * 2 + half
                        for j in range(4):
                            nc.tensor.transpose(
                                pt[:, half * 512 + j * 128:half * 512 + (j + 1) * 128],
                                xg[:, j, ci * 128:(ci + 1) * 128],
                                ident[:, :],
                            )
                    nc.vector.tensor_copy(
                        xt[:, cpair * 2:(cpair + 1) * 2, :],
                        pt[:, :].rearrange("p (a b) -> p a b", a=2),
                    )
                return xt

            for g in range(8):
                xt = xpose_group(xq, g)
                pq = ps_s.tile([128, 1024], f32, tag="s")
                for ci in range(4):
                    nc.tensor.matmul(
                        pq[:, 0:512], wqT[:, ci, :], xt[:, ci, :],
                        start=(ci == 0), stop=(ci == 3),
                    )
                nc.vector.tensor_scalar_add(
                    qT[:, g * 512:(g + 1) * 512], pq[:, 0:512], bsb["bq"][:, 0:1]
                )
            for g in range(8):
                xt = xpose_group(xk, g)
                pk = ps_s.tile([128, 1024], f32, tag="s")
                for ci in range(4):
                    nc.tensor.matmul(
                        pk[:, 0:512], wkT[:, ci, :], xt[:, ci, :],
                        start=(ci == 0), stop=(ci == 3),
                    )
                nc.vector.tensor_scalar_add(
                    kT[:, g * 512:(g + 1) * 512], pk[:, 0:512], bsb["bk"][:, 0:1]
                )
            for g in range(8):
                xt = xpose_group(xv, g)
                for j in range(4):
                    kc = g * 4 + j
                    pv = ps_s.tile([128, 1024], f32, tag="s")
                    for ci in range(4):
                        nc.tensor.matmul(
                            pv[:, 0:128],
                            xt[:, ci, j * 128:(j + 1) * 128],
                            wvT[:, ci, :],
                            start=(ci == 0), stop=(ci == 3),
                        )
                    nc.scalar.activation(
                        vsb[:, kc, :, 0:64],
                        pv[:, 0:128].rearrange("p (h d) -> p h d", h=2),
                        AF.Copy,
                    )

            # ---- main attention loop ----
            for qt in range(QTILES):
                q0 = qt * 512
                for hh in range(2):
                    po = hh * 64
                    es_tiles = []
                    for pair in range(KC // 2):
                        sp = ps_s.tile([128, 1024], f32, tag="s")
                        for m in range(2):
                            kc = pair * 2 + m
                            nc.tensor.matmul(
                                sp[:, m * 512:(m + 1) * 512],
                                kT[po:po + 64, kc * 128:(kc + 1) * 128],
                                qT[po:po + 64, q0:q0 + 512],
                                start=True, stop=True,
                            )
                        es = esp.tile([128, 1024], bf16, tag="es")
                        nc.scalar.activation(es[:, :], sp[:, :], AF.Exp, scale=0.125)
                        es_tiles.append(es)
                    # attn @ v_aug -> y^T (rows 0:64) + rowsums (row 64)
                    yp = ps_y.tile([65, 512], f32, tag="y")
                    for kc in range(KC):
                        nc.tensor.matmul(
                            yp[:, :],
                            vsb[:, kc, hh, :],
                            es_tiles[kc // 2][:, (kc % 2) * 512:(kc % 2 + 1) * 512],
                            start=(kc == 0), stop=(kc == KC - 1),
                        )
                    rc = smp.tile([1, 512], f32r, tag="rc")
                    with nc.allow_low_precision(reason="f32r rounding of softmax denominators is fine"):
                        nc.vector.reciprocal(rc[:, :], yp[64:65, :])
                    pb = ps_m.tile([128, 512], f32, tag="m")
                    nc.tensor.matmul(
                        pb[0:64, :], ones64[:, :], rc[:, :], start=True, stop=True
                    )
                    rb = smp.tile([64, 512], f32, tag="rb")
                    nc.vector.tensor_copy(rb[:, :], pb[0:64, :])
                    ytmp = smp.tile([64, 512], f32, tag="ytmp")
                    nc.vector.tensor_mul(ytmp[:, :], yp[0:64, :], rb[:, :])
                    nc.vector.tensor_scalar_add(
                        yT[po:po + 64, q0:q0 + 512], ytmp[:, :],
                        bsb["bv"][po:po + 64, 0:1],
                    )
                    # per-row reciprocals [128, 4]
                    pr = ps_m.tile([128, 512], f32, tag="m")
                    for s in range(4):
                        nc.tensor.transpose(
                            pr[:, s * 64:(s + 1) * 64],
                            rb[:, s * 128:(s + 1) * 128],
                            ident[0:64, 0:64],
                        )
                    rt = smp.tile([128, 4, 1], f32, tag="rt")
                    nc.vector.tensor_copy(
                        rt[:, :, :],
                        pr[:, 0:256].rearrange("p (s c) -> p s c", s=4)[:, :, 0:1],
                    )
                    # transpose + normalize + store attn rows
                    for s in range(4):
                        asb = ap_.tile([128, N], bf16, tag="asb")
                        for g2 in range(4):
                            pt2 = ps_t.tile([128, 8, 128], bf16, tag="t")
                            for kk in range(8):
                                kc = g2 * 8 + kk
                                nc.tensor.transpose(
                                    pt2[:, kk, :],
                                    es_tiles[kc // 2][
                                        :, (kc % 2) * 512 + s * 128:
                                        (kc % 2) * 512 + (s + 1) * 128
                                    ],
                                    identb[:, :],
                                )
                            nc.vector.tensor_scalar_mul(
                                asb[:, g2 * 1024:(g2 + 1) * 1024].rearrange(
                                    "p (a c) -> p a c", a=8
                                ),
                                pt2[:, :, :],
                                rt[:, s, :],
                            )
                        nc.gpsimd.dma_start(
                            out=attn_d[hh, q0 + s * 128:q0 + (s + 1) * 128, :],
                            in_=asb[:, :],
                        )
                # partial out projection for this qtile
                for rtile in range(4):
                    r0 = q0 + rtile * 128
                    pp = ps_m.tile([128, 512], f32, tag="m")
                    nc.tensor.matmul(
                        pp[:, :], yT[:, r0:r0 + 128], woT[:, :, :],
                        start=True, stop=True,
                    )
                    osb = otp.tile([128, C], f32, tag="o")
                    nc.vector.tensor_copy(osb[:, :], pp[:, :])
                    nc.sync.dma_start(out=out_d[r0:r0 + 128, :], in_=osb[:, :])

    nc.compile()
    return nc


_NC_CACHE = None


def _get_nc():
    global _NC_CACHE
    if _NC_CACHE is None:
        _NC_CACHE = build()
    return _NC_CACHE


def shard_inputs(query, key_t, value, Wq, bq, Wk, bk, Wv, bv, Wo, bo):
    asf = lambda a: np.ascontiguousarray(np.asarray(a, dtype=np.float32))
    query, key_t, value = asf(query), asf(key_t), asf(value)
    Wq, Wk, Wv, Wo = asf(Wq), asf(Wk), asf(Wv), asf(Wo)
    bq, bk, bv = asf(bq), asf(bk), asf(bv)
    in_maps = []
    for core in range(8):
        b, hp = core // 4, core % 4
        sl = slice(hp * 128, (hp + 1) * 128)
        in_maps.append({
            "xq": query[b], "xk": key_t[b], "xv": value[b],
            "wq": asf(Wq[sl]), "wk": asf(Wk[sl]), "wv": asf(Wv[sl]),
            "wo": asf(Wo[:, sl]),
            "bq": bq.reshape(1, C)[:, sl], "bk": bk.reshape(1, C)[:, sl],
            "bv": bv.reshape(1, C)[:, sl],
        })
    return in_maps


def assemble(results, bo):
    out = np.empty((B, N, C), np.float32)
    attn = np.empty((B, H, N, N), np.float32)
    for b in range(B):
        cores = [4 * b + i for i in range(4)]
        acc = results[cores[0]]["out"].copy()
        for c in cores[1:]:
            acc += results[c]["out"]
        out[b] = acc + np.asarray(bo, np.float32)[None, :]
        for i, c in enumerate(cores):
            attn[b, 2 * i:2 * i + 2] = results[c]["attn"]
    return out, attn


def kernel(**inputs):
    nc = _get_nc()
    in_maps = shard_inputs(**inputs)
    res = run_bass_kernel_spmd(nc, in_maps, core_ids=list(range(8)))
    return assemble(res.results, inputs["bo"])


# revision 9
# speedup vs baseline: 1.1233x; 1.1233x over previous
"""Multi-head attention (B=2, N=4096, C=512, H=8) on 8 TRN2 NeuronCores.

Sharding (data + head/tensor parallel, per the problem hint): core i handles
batch b = i//4 and head pair {2*(i%4), 2*(i%4)+1}. Projection weights are
split column-wise over heads (q/k/v) and row-wise for the out projection, so
each core computes its heads' full [N, N] score block, writes its attn slab,
and produces a partial out-projection that the host sum-reduces (+bo) during
unshard.

Per-core pipeline (all matmuls f32r unless noted):
  x -> PE-transpose -> xT chunks -> q/k/v projections
  scores S^T[kpos, qrow] = kT.T @ qT per 128-kpos chunk
  ACT exp (scale=1/8) PSUM -> SBUF bf16 (chunk pairs)
  attn @ v_aug (v with ones column, bf16) -> y^T + rowsums in PSUM
  recip -> PE broadcast -> y^T normalize (+bv)
  PE-transpose exp chunks -> natural [qrow, kpos] bf16 PSUM (8 per bank)
  DVE tensor_scalar normalize -> SBUF bf16 -> cast-DMA (bf16->f32) to DRAM
  out partial = y^T.T @ woT
"""

import numpy as np

import concourse.bass as bass
import concourse.bacc as bacc
import concourse.mybir as mybir
from concourse import tile
from concourse.bass_utils import run_bass_kernel_spmd
from concourse.masks import make_identity

f32 = mybir.dt.float32
f32r = mybir.dt.float32r
bf16 = mybir.dt.bfloat16
AF = mybir.ActivationFunctionType

B, N, C = 2, 4096, 512
H, D = 8, 64
KC = N // 128          # 32 kpos chunks
QTILES = N // 512      # 8 query tiles of 512 rows


def build():
    nc = bacc.Bacc("TRN2", target_bir_lowering=False, debug=False, num_devices=8)

    xq = nc.dram_tensor("xq", [N, C], f32, kind="ExternalInput").ap()
    xk = nc.dram_tensor("xk", [N, C], f32, kind="ExternalInput").ap()
    xv = nc.dram_tensor("xv", [N, C], f32, kind="ExternalInput").ap()
    # wq/wk/wv: [128, C] row slices of the weight (this core's 2 heads).
    # wo: [C, 128] column slice.  biases: [1, 128] slices (bo handled on host).
    wq = nc.dram_tensor("wq", [128, C], f32, kind="ExternalInput").ap()
    wk = nc.dram_tensor("wk", [128, C], f32, kind="ExternalInput").ap()
    wv = nc.dram_tensor("wv", [128, C], f32, kind="ExternalInput").ap()
    wo = nc.dram_tensor("wo", [C, 128], f32, kind="ExternalInput").ap()
    bq = nc.dram_tensor("bq", [1, 128], f32, kind="ExternalInput").ap()
    bk = nc.dram_tensor("bk", [1, 128], f32, kind="ExternalInput").ap()
    bv = nc.dram_tensor("bv", [1, 128], f32, kind="ExternalInput").ap()
    attn_d = nc.dram_tensor("attn", [2, N, N], f32, kind="ExternalOutput").ap()
    out_d = nc.dram_tensor("out", [N, C], f32, kind="ExternalOutput").ap()

    with tile.TileContext(nc) as tc:
        with (
            tc.tile_pool(name="const", bufs=1) as const,
            tc.tile_pool(name="wt", bufs=1) as wtp,
            tc.tile_pool(name="xg", bufs=2) as xgp,
            tc.tile_pool(name="xt", bufs=2) as xtp,
            tc.tile_pool(name="big", bufs=1) as bigp,
            tc.tile_pool(name="es", bufs=22) as esp,
            tc.tile_pool(name="asb", bufs=1) as ap_,
            tc.tile_pool(name="sm", bufs=3) as smp,
            tc.tile_pool(name="ot", bufs=2) as otp,
            tc.tile_pool(name="ps_s", bufs=2, space="PSUM") as ps_s,
            tc.tile_pool(name="ps_y", bufs=1, space="PSUM") as ps_y,
            tc.tile_pool(name="ps_t", bufs=2, space="PSUM") as ps_t,
            tc.tile_pool(name="ps_m", bufs=1, space="PSUM") as ps_m,
        ):
            # ---- constants ----
            ident = const.tile([128, 128], f32)
            make_identity(nc, ident[:, :])
            identb = const.tile([128, 128], bf16)
            nc.vector.tensor_copy(identb[:, :], ident[:, :])
            ones64f = const.tile([1, 64], f32, tag="o64f")
            nc.gpsimd.memset(ones64f[:, :], 1.0)
            ones64 = const.tile([1, 64], f32r, tag="o64r")
            nc.vector.tensor_copy(ones64[:, :], ones64f[:, :])

            # bias slices -> [128, 1] per-partition layout
            bsb = {}
            for name, bap in (("bq", bq), ("bk", bk), ("bv", bv)):
                t = const.tile([128, 1], f32, tag=name)
                nc.sync.dma_start(
                    out=t[:, :], in_=bap.rearrange("a (c p) -> p (a c)", p=128)
                )
                bsb[name] = t

            # ---- weight transposes ----
            # wqT/wkT: [p=c%128, ci, dk 128] f32r;  wvT: same;  woT: [p=ci, coj, 128]
            wqT = wtp.tile([128, 4, 128], f32r, tag="wqT")
            wkT = wtp.tile([128, 4, 128], f32r, tag="wkT")
            wvT = wtp.tile([128, 4, 128], f32r, tag="wvT")
            for wap, dst in ((wq, wqT), (wk, wkT), (wv, wvT)):
                t = xgp.tile([128, 4, 128], f32, tag="xg")
                nc.sync.dma_start(
                    out=t[:, :, :],
                    in_=wap.rearrange("p (cj c) -> p cj c", cj=4),
                )
                pt = ps_s.tile([128, 1024], f32, tag="s")
                for ci in range(4):
                    nc.tensor.transpose(
                        pt[:, ci * 128:(ci + 1) * 128],
                        t[:, ci, :],
                        ident[:, :],
                    )
                nc.vector.tensor_copy(
                    dst[:, :, :], pt[:, 0:512].rearrange("p (a b) -> p a b", a=4)
                )
            # woT[p=ci, coj, 128co]: transpose of wo[co, ci]
            wog = xgp.tile([128, 4, 128], f32, tag="xg")
            nc.sync.dma_start(
                out=wog[:, :, :],
                in_=wo.rearrange("(coj p) c -> p coj c", p=128),
            )
            woT = wtp.tile([128, 4, 128], f32r, tag="woT")
            pt = ps_s.tile([128, 1024], f32, tag="s")
            for coj in range(4):
                nc.tensor.transpose(
                    pt[:, coj * 128:(coj + 1) * 128], wog[:, coj, :], ident[:, :]
                )
            nc.vector.tensor_copy(
                woT[:, :, :], pt[:, 0:512].rearrange("p (a b) -> p a b", a=4)
            )

            # ---- x transposes + projections ----
            kT = bigp.tile([128, N], f32r, tag="kT")
            vsb = bigp.tile([128, KC, 2, 65], bf16, tag="v")
            nc.gpsimd.memset(vsb[:, :, :, 64:65], 1.0)
            qT = bigp.tile([128, N], f32r, tag="qT")
            yT = bigp.tile([128, N], f32r, tag="yT")

            def xpose_group(xap, g):
                """Transpose rows [512g, 512g+512) of x -> xt[p=c%128, ci, 512]."""
                xg = xgp.tile([128, 4, C], f32, tag="xg")
                nc.sync.dma_start(
                    out=xg[:, :, :],
                    in_=xap[g * 512:(g + 1) * 512, :].rearrange(
                        "(j p) c -> p j c", p=128
                    ),
                )
                xt = xtp.tile([128, 4, 512], f32r, tag="xt")
                for cpair in range(2):
                    pt = ps_s.tile([128, 1024], f32, tag="s")
                    for half in range(2):
                        ci = cpair * 2 + half
                        for j in range(4):
                            nc.tensor.transpose(
                                pt[:, half * 512 + j * 128:half * 512 + (j + 1) * 128],
                                xg[:, j, ci * 128:(ci + 1) * 128],
                                ident[:, :],
                            )
                    nc.vector.tensor_copy(
                        xt[:, cpair * 2:(cpair + 1) * 2, :],
                        pt[:, :].rearrange("p (a b) -> p a b", a=2),
                    )
                return xt

            for g in range(8):
                xt = xpose_group(xq, g)
                pq = ps_s.tile([128, 1024], f32, tag="s")
                for ci in range(4):
                    nc.tensor.matmul(
                        pq[:, 0:512], wqT[:, ci, :], xt[:, ci, :],
                        start=(ci == 0), stop=(ci == 3),
                    )
                nc.vector.tensor_scalar_add(
                    qT[:, g * 512:(g + 1) * 512], pq[:, 0:512], bsb["bq"][:, 0:1]
                )
            for g in range(8):
                xt = xpose_group(xk, g)
                pk = ps_s.tile([128, 1024], f32, tag="s")
                for ci in range(4):
                    nc.tensor.matmul(
                        pk[:, 0:512], wkT[:, ci, :], xt[:, ci, :],
                        start=(ci == 0), stop=(ci == 3),
                    )
                nc.vector.tensor_scalar_add(
                    kT[:, g * 512:(g + 1) * 512], pk[:, 0:512], bsb["bk"][:, 0:1]
                )
            for g in range(8):
                xt = xpose_group(xv, g)
                for j in range(4):
                    kc = g * 4 + j
                    pv = ps_s.tile([128, 1024], f32, tag="s")
                    for ci in range(4):
                        nc.tensor.matmul(
                            pv[:, 0:128],
                            xt[:, ci, j * 128:(j + 1) * 128],
                            wvT[:, ci, :],
                            start=(ci == 0), stop=(ci == 3),
                        )
                    nc.scalar.activation(
                        vsb[:, kc, :, 0:64],
                        pv[:, 0:128].rearrange("p (h d) -> p h d", h=2),
                        AF.Copy,
                    )

            # ---- main attention loop (software-pipelined) ----
            # Iteration (qt, hh) computes scores+exp+av for its block; the
            # PE transposes + DVE normalize + attn DMAs for iteration i run
            # interleaved with iteration i+1's score matmuls so the PE never
            # sees a long matmul-free stretch (keeps the HAM clock at 2.4GHz).
            def tp_norm_group(es_list, rt, asb_t, p):
                s, g2 = p % 4, p // 4
                pt2 = ps_t.tile([128, 8, 128], bf16, tag="t")
                for kk in range(8):
                    kc = g2 * 8 + kk
                    nc.tensor.transpose(
                        pt2[:, kk, :],
                        es_list[kc // 2][
                            :, (kc % 2) * 512 + s * 128:(kc % 2) * 512 + (s + 1) * 128
                        ],
                        identb[:, :],
                    )
                nc.vector.tensor_scalar_mul(
                    asb_t[:, s, g2 * 1024:(g2 + 1) * 1024].rearrange(
                        "pp (a c) -> pp a c", a=8
                    ),
                    pt2[:, :, :],
                    rt[:, s, :],
                )

            def attn_dmas(asb_t, hh, q0):
                for s in range(4):
                    nc.gpsimd.dma_start(
                        out=attn_d[hh, q0 + s * 128:q0 + (s + 1) * 128, :],
                        in_=asb_t[:, s, :],
                    )

            prev = None
            for qt in range(QTILES):
                q0 = qt * 512
                for hh in range(2):
                    po = hh * 64
                    if prev is not None:
                        asb_p = ap_.tile([128, 4, N], bf16, tag="asb")
                    es_tiles = []
                    for p in range(KC // 2):
                        sp = ps_s.tile([128, 1024], f32, tag="s")
                        for m in range(2):
                            kc = p * 2 + m
                            nc.tensor.matmul(
                                sp[:, m * 512:(m + 1) * 512],
                                kT[po:po + 64, kc * 128:(kc + 1) * 128],
                                qT[po:po + 64, q0:q0 + 512],
                                start=True, stop=True,
                            )
                        es = esp.tile([128, 1024], bf16, tag="es")
                        nc.scalar.activation(es[:, :], sp[:, :], AF.Exp, scale=0.125)
                        es_tiles.append(es)
                        if prev is not None:
                            tp_norm_group(prev[0], prev[1], asb_p, p)
                    if prev is not None:
                        attn_dmas(asb_p, prev[2], prev[3])
                    # attn @ v_aug -> y^T (rows 0:64) + rowsums (row 64)
                    yp = ps_y.tile([65, 512], f32, tag="y")
                    for kc in range(KC):
                        nc.tensor.matmul(
                            yp[:, :],
                            vsb[:, kc, hh, :],
                            es_tiles[kc // 2][:, (kc % 2) * 512:(kc % 2 + 1) * 512],
                            start=(kc == 0), stop=(kc == KC - 1),
                        )
                    rc = smp.tile([1, 512], f32r, tag="rc")
                    with nc.allow_low_precision(reason="f32r softmax denominators"):
                        nc.vector.reciprocal(rc[:, :], yp[64:65, :])
                    pb = ps_m.tile([128, 512], f32, tag="m")
                    nc.tensor.matmul(
                        pb[0:64, :], ones64[:, :], rc[:, :], start=True, stop=True
                    )
                    rb = smp.tile([64, 512], f32, tag="rb")
                    nc.vector.tensor_copy(rb[:, :], pb[0:64, :])
                    ytmp = smp.tile([64, 512], f32, tag="ytmp")
                    nc.vector.tensor_mul(ytmp[:, :], yp[0:64, :], rb[:, :])
                    nc.vector.tensor_scalar_add(
                        yT[po:po + 64, q0:q0 + 512], ytmp[:, :],
                        bsb["bv"][po:po + 64, 0:1],
                    )
                    # per-row reciprocals [128, 4, 1]
                    pr = ps_m.tile([128, 512], f32, tag="m")
                    for s in range(4):
                        nc.tensor.transpose(
                            pr[:, s * 64:(s + 1) * 64],
                            rb[:, s * 128:(s + 1) * 128],
                            ident[0:64, 0:64],
                        )
                    rt = smp.tile([128, 4, 1], f32, tag="rt")
                    nc.vector.tensor_copy(
                        rt[:, :, :],
                        pr[:, 0:256].rearrange("p (s c) -> p s c", s=4)[:, :, 0:1],
                    )
                    prev = (es_tiles, rt, hh, q0)
                # partial out projection for this qtile
                for rtile in range(4):
                    r0 = q0 + rtile * 128
                    pp = ps_m.tile([128, 512], f32, tag="m")
                    nc.tensor.matmul(
                        pp[:, :], yT[:, r0:r0 + 128], woT[:, :, :],
                        start=True, stop=True,
                    )
                    osb = otp.tile([128, C], f32, tag="o")
                    nc.vector.tensor_copy(osb[:, :], pp[:, :])
                    nc.sync.dma_start(out=out_d[r0:r0 + 128, :], in_=osb[:, :])
            # drain: transposes/norm/DMA for the final iteration
            asb_p = ap_.tile([128, 4, N], bf16, tag="asb")
            for p in range(KC // 2):
                tp_norm_group(prev[0], prev[1], asb_p, p)
            attn_dmas(asb_p, prev[2], prev[3])

    nc.compile()
    return nc


_NC_CACHE = None


def _get_nc():
    global _NC_CACHE
    if _NC_CACHE is None:
        _NC_CACHE = build()
    return _NC_CACHE


def shard_inputs(query, key_t, value, Wq, bq, Wk, bk, Wv, bv, Wo, bo):
    asf = lambda a: np.ascontiguousarray(np.asarray(a, dtype=np.float32))
    query, key_t, value = asf(query), asf(key_t), asf(value)
    Wq, Wk, Wv, Wo = asf(Wq), asf(Wk), asf(Wv), asf(Wo)
    bq, bk, bv = asf(bq), asf(bk), asf(bv)
    in_maps = []
    for core in range(8):
        b, hp = core // 4, core % 4
        sl = slice(hp * 128, (hp + 1) * 128)
        in_maps.append({
            "xq": query[b], "xk": key_t[b], "xv": value[b],
            "wq": asf(Wq[sl]), "wk": asf(Wk[sl]), "wv": asf(Wv[sl]),
            "wo": asf(Wo[:, sl]),
            "bq": bq.reshape(1, C)[:, sl], "bk": bk.reshape(1, C)[:, sl],
            "bv": bv.reshape(1, C)[:, sl],
        })
    return in_maps


def assemble(results, bo):
    out = np.empty((B, N, C), np.float32)
    attn = np.empty((B, H, N, N), np.float32)
    for b in range(B):
        cores = [4 * b + i for i in range(4)]
        acc = results[cores[0]]["out"].copy()
        for c in cores[1:]:
            acc += results[c]["out"]
        out[b] = acc + np.asarray(bo, np.float32)[None, :]
        for i, c in enumerate(cores):
            attn[b, 2 * i:2 * i + 2] = results[c]["attn"]
    return out, attn


def kernel(**inputs):
    nc = _get_nc()
    in_maps = shard_inputs(**inputs)
    res = run_bass_kernel_spmd(nc, in_maps, core_ids=list(range(8)))
    return assemble(res.results, inputs["bo"])


# revision 11
# speedup vs baseline: 1.1867x; 1.0564x over previous
"""Multi-head attention (B=2, N=4096, C=512, H=8) on 8 TRN2 NeuronCores.

Sharding (data + head/tensor parallel, per the problem hint): core i handles
batch b = i//4 and head pair {2*(i%4), 2*(i%4)+1}. Projection weights are
split column-wise over heads (q/k/v) and row-wise for the out projection, so
each core computes its heads' full [N, N] score block, writes its attn slab,
and produces a partial out-projection that the host sum-reduces (+bo) during
unshard.

Per-core pipeline (all matmuls f32r unless noted):
  x -> PE-transpose -> xT chunks -> q/k/v projections
  scores S^T[kpos, qrow] = kT.T @ qT per 128-kpos chunk
  ACT exp (scale=1/8) PSUM -> SBUF bf16 (chunk pairs)
  attn @ v_aug (v with ones column, bf16) -> y^T + rowsums in PSUM
  recip -> PE broadcast -> y^T normalize (+bv)
  PE-transpose exp chunks -> natural [qrow, kpos] bf16 PSUM (8 per bank)
  DVE tensor_scalar normalize -> SBUF bf16 -> cast-DMA (bf16->f32) to DRAM
  out partial = y^T.T @ woT
"""

import numpy as np

import concourse.bass as bass
import concourse.bacc as bacc
import concourse.mybir as mybir
from concourse import tile
from concourse.bass_utils import run_bass_kernel_spmd
from concourse.masks import make_identity

f32 = mybir.dt.float32
f32r = mybir.dt.float32r
bf16 = mybir.dt.bfloat16
AF = mybir.ActivationFunctionType

B, N, C = 2, 4096, 512
H, D = 8, 64
KC = N // 128          # 32 kpos chunks
QTILES = N // 512      # 8 query tiles of 512 rows


def build():
    nc = bacc.Bacc("TRN2", target_bir_lowering=False, debug=False, num_devices=8)

    xq = nc.dram_tensor("xq", [N, C], f32, kind="ExternalInput").ap()
    xk = nc.dram_tensor("xk", [N, C], f32, kind="ExternalInput").ap()
    xv = nc.dram_tensor("xv", [N, C], f32, kind="ExternalInput").ap()
    # wq/wk/wv: [128, C] row slices of the weight (this core's 2 heads).
    # wo: [C, 128] column slice.  biases: [1, 128] slices (bo handled on host).
    wq = nc.dram_tensor("wq", [128, C], f32, kind="ExternalInput").ap()
    wk = nc.dram_tensor("wk", [128, C], f32, kind="ExternalInput").ap()
    wv = nc.dram_tensor("wv", [128, C], f32, kind="ExternalInput").ap()
    wo = nc.dram_tensor("wo", [C, 128], f32, kind="ExternalInput").ap()
    bq = nc.dram_tensor("bq", [1, 128], f32, kind="ExternalInput").ap()
    bk = nc.dram_tensor("bk", [1, 128], f32, kind="ExternalInput").ap()
    bv = nc.dram_tensor("bv", [1, 128], f32, kind="ExternalInput").ap()
    attn_d = nc.dram_tensor("attn", [2, N, N], f32, kind="ExternalOutput").ap()
    out_d = nc.dram_tensor("out", [N, C], f32, kind="ExternalOutput").ap()

    with tile.TileContext(nc) as tc:
        with (
            tc.tile_pool(name="const", bufs=1) as const,
            tc.tile_pool(name="wt", bufs=1) as wtp,
            tc.tile_pool(name="xg", bufs=2) as xgp,
            tc.tile_pool(name="xt", bufs=2) as xtp,
            tc.tile_pool(name="big", bufs=1) as bigp,
            tc.tile_pool(name="es", bufs=22) as esp,
            tc.tile_pool(name="asb", bufs=1) as ap_,
            tc.tile_pool(name="sm", bufs=3) as smp,
            tc.tile_pool(name="ot", bufs=2) as otp,
            tc.tile_pool(name="ps_s", bufs=2, space="PSUM") as ps_s,
            tc.tile_pool(name="ps_y", bufs=1, space="PSUM") as ps_y,
            tc.tile_pool(name="ps_t", bufs=2, space="PSUM") as ps_t,
            tc.tile_pool(name="ps_m", bufs=1, space="PSUM") as ps_m,
        ):
            # ---- constants ----
            ident = const.tile([128, 128], f32)
            make_identity(nc, ident[:, :])
            identb = const.tile([128, 128], bf16)
            nc.vector.tensor_copy(identb[:, :], ident[:, :])
            ones64f = const.tile([1, 64], f32, tag="o64f")
            nc.gpsimd.memset(ones64f[:, :], 1.0)
            ones64 = const.tile([1, 64], f32r, tag="o64r")
            nc.vector.tensor_copy(ones64[:, :], ones64f[:, :])

            # bias slices -> [128, 1] per-partition layout
            bsb = {}
            for name, bap in (("bq", bq), ("bk", bk), ("bv", bv)):
                t = const.tile([128, 1], f32, tag=name)
                nc.sync.dma_start(
                    out=t[:, :], in_=bap.rearrange("a (c p) -> p (a c)", p=128)
                )
                bsb[name] = t

            # ---- weight transposes ----
            # wqT/wkT: [p=c%128, ci, dk 128] f32r;  wvT: same;  woT: [p=ci, coj, 128]
            wqT = wtp.tile([128, 4, 128], bf16, tag="wqT")
            wkT = wtp.tile([128, 4, 128], bf16, tag="wkT")
            wvT = wtp.tile([128, 4, 128], bf16, tag="wvT")
            for wap, dst in ((wq, wqT), (wk, wkT), (wv, wvT)):
                t = xgp.tile([128, 4, 128], bf16, tag="xg")
                nc.gpsimd.dma_start(
                    out=t[:, :, :],
                    in_=wap.rearrange("p (cj c) -> p cj c", cj=4),
                )
                pt = ps_s.tile([128, 1024], bf16, tag="s")
                for ci in range(4):
                    nc.tensor.transpose(
                        pt[:, ci * 128:(ci + 1) * 128],
                        t[:, ci, :],
                        identb[:, :],
                    )
                nc.vector.tensor_copy(
                    dst[:, :, :], pt[:, 0:512].rearrange("p (a b) -> p a b", a=4)
                )
            # woT[p=ci, coj, 128co]: transpose of wo[co, ci]
            wog = xgp.tile([128, 4, 128], f32, tag="xg")
            nc.sync.dma_start(
                out=wog[:, :, :],
                in_=wo.rearrange("(coj p) c -> p coj c", p=128),
            )
            woT = wtp.tile([128, 4, 128], f32r, tag="woT")
            pt = ps_s.tile([128, 1024], f32, tag="s")
            for coj in range(4):
                nc.tensor.transpose(
                    pt[:, coj * 128:(coj + 1) * 128], wog[:, coj, :], ident[:, :]
                )
            nc.vector.tensor_copy(
                woT[:, :, :], pt[:, 0:512].rearrange("p (a b) -> p a b", a=4)
            )

            # ---- x transposes + projections ----
            kT = bigp.tile([128, N], bf16, tag="kT")
            vsb = bigp.tile([128, KC, 2, 65], bf16, tag="v")
            nc.gpsimd.memset(vsb[:, :, :, 64:65], 1.0)
            qT = bigp.tile([128, N], bf16, tag="qT")
            yT = bigp.tile([128, N], f32r, tag="yT")

            def xpose_group(xap, g):
                """Transpose rows [512g, 512g+512) of x -> xt[p=c%128, ci, 512]."""
                xg = xgp.tile([128, 4, C], bf16, tag="xg")
                nc.gpsimd.dma_start(
                    out=xg[:, :, :],
                    in_=xap[g * 512:(g + 1) * 512, :].rearrange(
                        "(j p) c -> p j c", p=128
                    ),
                )
                xt = xtp.tile([128, 4, 512], bf16, tag="xt")
                for cpair in range(2):
                    pt = ps_s.tile([128, 1024], bf16, tag="s")
                    for half in range(2):
                        ci = cpair * 2 + half
                        for j in range(4):
                            nc.tensor.transpose(
                                pt[:, half * 512 + j * 128:half * 512 + (j + 1) * 128],
                                xg[:, j, ci * 128:(ci + 1) * 128],
                                identb[:, :],
                            )
                    nc.vector.tensor_copy(
                        xt[:, cpair * 2:(cpair + 1) * 2, :],
                        pt[:, :].rearrange("p (a b) -> p a b", a=2),
                    )
                return xt

            for g in range(8):
                xt = xpose_group(xq, g)
                pq = ps_s.tile([128, 1024], f32, tag="s")
                for ci in range(4):
                    nc.tensor.matmul(
                        pq[:, 0:512], wqT[:, ci, :], xt[:, ci, :],
                        start=(ci == 0), stop=(ci == 3),
                    )
                nc.vector.tensor_scalar_add(
                    qT[:, g * 512:(g + 1) * 512], pq[:, 0:512], bsb["bq"][:, 0:1]
                )
            for g in range(8):
                xt = xpose_group(xk, g)
                pk = ps_s.tile([128, 1024], f32, tag="s")
                for ci in range(4):
                    nc.tensor.matmul(
                        pk[:, 0:512], wkT[:, ci, :], xt[:, ci, :],
                        start=(ci == 0), stop=(ci == 3),
                    )
                nc.vector.tensor_scalar_add(
                    kT[:, g * 512:(g + 1) * 512], pk[:, 0:512], bsb["bk"][:, 0:1]
                )
            for g in range(8):
                xt = xpose_group(xv, g)
                for j in range(4):
                    kc = g * 4 + j
                    pv = ps_s.tile([128, 1024], f32, tag="s")
                    for ci in range(4):
                        nc.tensor.matmul(
                            pv[:, 0:128],
                            xt[:, ci, j * 128:(j + 1) * 128],
                            wvT[:, ci, :],
                            start=(ci == 0), stop=(ci == 3),
                        )
                    nc.scalar.activation(
                        vsb[:, kc, :, 0:64],
                        pv[:, 0:128].rearrange("p (h d) -> p h d", h=2),
                        AF.Copy,
                    )

            # ---- main attention loop (software-pipelined) ----
            # Iteration (qt, hh) computes scores+exp+av for its block; the
            # PE transposes + DVE normalize + attn DMAs for iteration i run
            # interleaved with iteration i+1's score matmuls so the PE never
            # sees a long matmul-free stretch (keeps the HAM clock at 2.4GHz).
            def tp_norm_group(es_list, rt, asb_t, p):
                s, g2 = p % 4, p // 4
                pt2 = ps_t.tile([128, 8, 128], bf16, tag="t")
                for kk in range(8):
                    kc = g2 * 8 + kk
                    nc.tensor.transpose(
                        pt2[:, kk, :],
                        es_list[kc // 2][
                            :, (kc % 2) * 512 + s * 128:(kc % 2) * 512 + (s + 1) * 128
                        ],
                        identb[:, :],
                    )
                nc.vector.tensor_scalar_mul(
                    asb_t[:, s, g2 * 1024:(g2 + 1) * 1024].rearrange(
                        "pp (a c) -> pp a c", a=8
                    ),
                    pt2[:, :, :],
                    rt[:, s, :],
                )

            def attn_dmas(asb_t, hh, q0):
                for s in range(4):
                    nc.gpsimd.dma_start(
                        out=attn_d[hh, q0 + s * 128:q0 + (s + 1) * 128, :],
                        in_=asb_t[:, s, :],
                    )

            prev = None
            for qt in range(QTILES):
                q0 = qt * 512
                for hh in range(2):
                    po = hh * 64
                    if prev is not None:
                        asb_p = ap_.tile([128, 4, N], bf16, tag="asb")
                    es_tiles = []
                    for p in range(KC // 2):
                        sp = ps_s.tile([128, 1024], f32, tag="s")
                        for m in range(2):
                            kc = p * 2 + m
                            nc.tensor.matmul(
                                sp[:, m * 512:(m + 1) * 512],
                                kT[po:po + 64, kc * 128:(kc + 1) * 128],
                                qT[po:po + 64, q0:q0 + 512],
                                start=True, stop=True,
                            )
                        es = esp.tile([128, 1024], bf16, tag="es")
                        nc.scalar.activation(es[:, :], sp[:, :], AF.Exp, scale=0.125)
                        es_tiles.append(es)
                        if prev is not None:
                            tp_norm_group(prev[0], prev[1], asb_p, p)
                    if prev is not None:
                        attn_dmas(asb_p, prev[2], prev[3])
                    # attn @ v_aug -> y^T (rows 0:64) + rowsums (row 64)
                    yp = ps_y.tile([65, 512], f32, tag="y")
                    for kc in range(KC):
                        nc.tensor.matmul(
                            yp[:, :],
                            vsb[:, kc, hh, :],
                            es_tiles[kc // 2][:, (kc % 2) * 512:(kc % 2 + 1) * 512],
                            start=(kc == 0), stop=(kc == KC - 1),
                        )
                    rc = smp.tile([1, 512], f32r, tag="rc")
                    with nc.allow_low_precision(reason="f32r softmax denominators"):
                        nc.vector.reciprocal(rc[:, :], yp[64:65, :])
                    pb = ps_m.tile([128, 512], f32, tag="m")
                    nc.tensor.matmul(
                        pb[0:64, :], ones64[:, :], rc[:, :], start=True, stop=True
                    )
                    rb = smp.tile([64, 512], f32, tag="rb")
                    nc.vector.tensor_copy(rb[:, :], pb[0:64, :])
                    ytmp = smp.tile([64, 512], f32, tag="ytmp")
                    nc.vector.tensor_mul(ytmp[:, :], yp[0:64, :], rb[:, :])
                    nc.vector.tensor_scalar_add(
                        yT[po:po + 64, q0:q0 + 512], ytmp[:, :],
                        bsb["bv"][po:po + 64, 0:1],
                    )
                    # per-row reciprocals [128, 4, 1]
                    pr = ps_m.tile([128, 512], f32, tag="m")
                    for s in range(4):
                        nc.tensor.transpose(
                            pr[:, s * 64:(s + 1) * 64],
                            rb[:, s * 128:(s + 1) * 128],
                            ident[0:64, 0:64],
                        )
                    rt = smp.tile([128, 4, 1], f32, tag="rt")
                    nc.vector.tensor_copy(
                        rt[:, :, :],
                        pr[:, 0:256].rearrange("p (s c) -> p s c", s=4)[:, :, 0:1],
                    )
                    prev = (es_tiles, rt, hh, q0)
                # partial out projection for this qtile
                for rtile in range(4):
                    r0 = q0 + rtile * 128
                    pp = ps_m.tile([128, 512], f32, tag="m")
                    nc.tensor.matmul(
                        pp[:, :], yT[:, r0:r0 + 128], woT[:, :, :],
                        start=True, stop=True,
                    )
                    osb = otp.tile([128, C], f32, tag="o")
                    nc.vector.tensor_copy(osb[:, :], pp[:, :])
                    nc.sync.dma_start(out=out_d[r0:r0 + 128, :], in_=osb[:, :])
            # drain: transposes/norm/DMA for the final iteration
            asb_p = ap_.tile([128, 4, N], bf16, tag="asb")
            for p in range(KC // 2):
                tp_norm_group(prev[0], prev[1], asb_p, p)
            attn_dmas(asb_p, prev[2], prev[3])

    nc.compile()
    return nc


_NC_CACHE = None


def _get_nc():
    global _NC_CACHE
    if _NC_CACHE is None:
        _NC_CACHE = build()
    return _NC_CACHE


def shard_inputs(query, key_t, value, Wq, bq, Wk, bk, Wv, bv, Wo, bo):
    asf = lambda a: np.ascontiguousarray(np.asarray(a, dtype=np.float32))
    query, key_t, value = asf(query), asf(key_t), asf(value)
    Wq, Wk, Wv, Wo = asf(Wq), asf(Wk), asf(Wv), asf(Wo)
    bq, bk, bv = asf(bq), asf(bk), asf(bv)
    in_maps = []
    for core in range(8):
        b, hp = core // 4, core % 4
        sl = slice(hp * 128, (hp + 1) * 128)
        in_maps.append({
            "xq": query[b], "xk": key_t[b], "xv": value[b],
            "wq": asf(Wq[sl]), "wk": asf(Wk[sl]), "wv": asf(Wv[sl]),
            "wo": asf(Wo[:, sl]),
            "bq": bq.reshape(1, C)[:, sl], "bk": bk.reshape(1, C)[:, sl],
            "bv": bv.reshape(1, C)[:, sl],
        })
    return in_maps


def assemble(results, bo):
    out = np.empty((B, N, C), np.float32)
    attn = np.empty((B, H, N, N), np.float32)
    for b in range(B):
        cores = [4 * b + i for i in range(4)]
        acc = results[cores[0]]["out"].copy()
        for c in cores[1:]:
            acc += results[c]["out"]
        out[b] = acc + np.asarray(bo, np.float32)[None, :]
        for i, c in enumerate(cores):
            attn[b, 2 * i:2 * i + 2] = results[c]["attn"]
    return out, attn


def kernel(**inputs):
    nc = _get_nc()
    in_maps = shard_inputs(**inputs)
    res = run_bass_kernel_spmd(nc, in_maps, core_ids=list(range(8)))
    return assemble(res.results, inputs["bo"])
